# revision 2
# baseline (speedup 1.0000x reference)
"""TRN2 Bass kernel for nn_Decoder_25615184954195 (2-layer LSTM decoder, 32 steps).

Strategy: data-parallel over batch (8 cores x 32 rows), full sequential loop
local per core.  All 32 weight k-tiles are SBUF-resident in fp8 e3m4 (16 MB
per core), eliminating the per-step HBM weight streaming that bound the bf16
version.  Weights are pre-scaled by 128 into e3m4's normal range; the
stationary activations carry the compensating 1/128, folded for free into the
PSUM->SBUF copy after each h transpose (tensor_scalar_mul) and into the host
prep of x0/h0/W_lin.  PSUM accumulates in fp32 so gate values are exact up to
the e3m4 weight quantization (~2.4e-3 end-to-end rel l2).

Per-core layouts (unchanged from the bf16 version):
  - Contraction K = 2048 = [x(1024) | h(1024)], 16 k-tiles of 128.
  - Stationary operand: xhT k-tile [128, 32] bf16 (batch in the free dim).
  - Moving operand: column-permuted weight k-tile [128, 4096] fp8e3; the
    4-way column-tiled matmul's col-group j computes H-quarter j for all 4
    gates: psum[32*j + b, 256*G + h2] = gates[b, 1024*G + 256*j + h2].
  - Folded elementwise layout: partitions = (H-quarter, batch) so gate math
    runs on all 128 lanes.  c stays fp32.  h is rebuilt transposed via two
    PE transposes, deferred into the next layer-step between its h-rounds and
    x-rounds so the PE fills the gate-chain latency with matmuls.
"""
import sys

for _p in ("/opt/trn_rl_repo", "/root/.axon_site/_ro/trn_rl_repo"):
    if _p not in sys.path:
        sys.path.insert(0, _p)

import numpy as np
import ml_dtypes

import concourse.bacc as bacc
import concourse.mybir as mybir
import concourse.tile as tile
from concourse import masks
from concourse import bass_utils

BF16 = mybir.dt.bfloat16
FP8 = mybir.dt.float8e3
F32 = mybir.dt.float32
AF = mybir.ActivationFunctionType

N_CORES = 8
B = 32            # batch rows per core (256 total)
KT = 16           # k-tiles per layer
L = 2
GCOLS = 4096
NUM_STEPS = 32
SCALE = 128.0     # weights *SCALE in fp8; stationary activations /SCALE

ROUNDS = list(range(8, 16)) + list(range(0, 8))


def _build(num_steps=NUM_STEPS, reps=1):
    # weight tiles ordered by first use so step 0 starts ASAP
    res_list = [(l, k) for l in range(L) for k in ROUNDS]

    nc = bacc.Bacc("TRN2", target_bir_lowering=False, debug=False,
                   num_devices=N_CORES)

    d_w = nc.dram_tensor("wres", [len(res_list), 128, GCOLS], FP8,
                         kind="ExternalInput")
    d_bias = nc.dram_tensor("biasf", [L, 128, 1024], BF16, kind="ExternalInput")
    d_x0t = nc.dram_tensor("x0t", [128, 8 * B], BF16, kind="ExternalInput")
    d_h0t = nc.dram_tensor("h0t", [L, 2, 128, 128], BF16, kind="ExternalInput")
    d_c0 = nc.dram_tensor("c0f", [L, 128, 256], F32, kind="ExternalInput")
    d_wlin = nc.dram_tensor("wlint", [128, 8 * 512], BF16, kind="ExternalInput")
    d_blin = nc.dram_tensor("blinr", [B, 512], F32, kind="ExternalInput")
    d_y = nc.dram_tensor("y", [B, 512], F32, kind="ExternalOutput")

    with tile.TileContext(nc) as tc:
        import contextlib
        with contextlib.ExitStack() as ctx:
            sb = ctx.enter_context(tc.tile_pool(name="sb", bufs=1))
            sb2 = ctx.enter_context(tc.tile_pool(name="sb2", bufs=2))
            psum = ctx.enter_context(tc.tile_pool(name="psum", bufs=2, space="PSUM"))
            psum1 = ctx.enter_context(tc.tile_pool(name="psum1", bufs=2, space="PSUM"))
            psumgb = ctx.enter_context(tc.tile_pool(name="psumgb", bufs=1, space="PSUM"))

            w_sb = {}
            for i, (l, k) in enumerate(res_list):
                t = sb.tile([128, GCOLS], FP8, name=f"w_{l}_{k}")
                nc.sync.dma_start(t[:], d_w[i])
                w_sb[(l, k)] = t
            bias_sb = sb.tile([128, L * 1024], BF16, name="bias_sb")
            for l in range(L):
                nc.sync.dma_start(bias_sb[:, 1024 * l:1024 * (l + 1)], d_bias[l])
            x0t_sb = sb.tile([128, 8 * B], BF16, name="x0t_sb")
            nc.sync.dma_start(x0t_sb[:], d_x0t[:])
            hT = [[sb.tile([128, 128], BF16, name=f"hT_{l}_{hf}") for hf in range(2)]
                  for l in range(L)]
            for l in range(L):
                for hf in range(2):
                    nc.sync.dma_start(hT[l][hf][:], d_h0t[l, hf])
            c_sb = [sb.tile([128, 256], F32, name=f"c_{l}") for l in range(L)]
            for l in range(L):
                nc.sync.dma_start(c_sb[l][:], d_c0[l])
            wlin_sb = sb.tile([128, 8 * 512], BF16, name="wlin_sb")
            nc.sync.dma_start(wlin_sb[:], d_wlin[:])
            ident = sb.tile([128, 128], BF16, name="ident")
            masks.make_identity(nc, ident[:])

            def stat_ap(l, k, t):
                if k >= 8:
                    kh = k - 8
                    return hT[l][kh % 2][:, 32 * (kh // 2):32 * (kh // 2) + 32]
                if l == 0:
                    if t == 0:
                        return x0t_sb[:, 32 * k:32 * (k + 1)]
                    src = hT[1]
                else:
                    src = hT[0]
                return src[k % 2][:, 32 * (k // 2):32 * (k // 2) + 32]

            pending_tr = []   # deferred transposes: (hbf tile, target layer)

            def flush_transposes():
                while pending_tr:
                    hbf_p, l_p = pending_tr.pop(0)
                    for hf in range(2):
                        pt = psum1.tile([128, 128], BF16, name="pt", tag="pt")
                        nc.tensor.transpose(pt[:], hbf_p[:, 128 * hf:128 * (hf + 1)],
                                            ident[:])
                        nc.vector.tensor_scalar_mul(hT[l_p][hf][:], pt[:], 1.0 / SCALE)

            for rep in range(reps):
                for t in range(num_steps):
                    tt = t if rep == 0 else 1  # steps past the first read hT, not x0
                    for l in range(L):
                        ps = psum.tile([128, 1024], F32, name="gps", tag="gps")
                        for ri, k in enumerate(ROUNDS):
                            if ri == 8:
                                flush_transposes()   # before x-rounds need hT
                            w = w_sb[(l, k)]
                            lhsT = stat_ap(l, k, tt)
                            for hv in range(2):      # PSUM bank limit: N<=512 fp32
                                for j in range(4):   # col-groups run concurrently
                                    nc.tensor.matmul(
                                        ps[32 * j:32 * (j + 1), 512 * hv:512 * (hv + 1)],
                                        lhsT,
                                        w[:, 1024 * j + 512 * hv:1024 * j + 512 * (hv + 1)],
                                        start=(ri == 0), stop=(ri == len(ROUNDS) - 1),
                                        tile_position=(0, 32 * j),
                                        skip_group_check=True,
                                    )

                        gb = psumgb.tile([128, 1024], F32, name="gb", tag="gb")
                        nc.vector.tensor_add(gb[:], ps[:],
                                             bias_sb[:, 1024 * l:1024 * (l + 1)])
                        s_if = sb.tile([128, 512], F32, name="s_if", tag="s_if")
                        t_g = sb.tile([128, 256], F32, name="t_g", tag="t_g")
                        s_o = sb.tile([128, 256], F32, name="s_o", tag="s_o")
                        nc.scalar.activation(s_if[:], gb[:, 0:512], AF.Sigmoid)
                        nc.scalar.activation(t_g[:], gb[:, 512:768], AF.Tanh)
                        nc.scalar.activation(s_o[:], gb[:, 768:1024], AF.Sigmoid)
                        t1 = sb.tile([128, 256], F32, name="t1", tag="t1")
                        t2 = sb.tile([128, 256], F32, name="t2", tag="t2")
                        nc.vector.tensor_mul(t1[:], s_if[:, 256:512], c_sb[l][:])
                        nc.vector.tensor_mul(t2[:], s_if[:, 0:256], t_g[:])
                        nc.vector.tensor_add(c_sb[l][:], t1[:], t2[:])
                        th = sb.tile([128, 256], F32, name="th", tag="t1")
                        nc.scalar.activation(th[:], c_sb[l][:], AF.Tanh)
                        hbf = sb2.tile([128, 256], BF16, name="hbf", tag="hbf")
                        nc.vector.tensor_mul(hbf[:], s_o[:], th[:])
                        pending_tr.append((hbf, l))

            flush_transposes()

            # ---- linear head: pred = h1 @ W_lin.T + b_lin ----
            blin_sb = sb.tile([B, 512], F32, name="blin_sb", tag="t1")
            nc.sync.dma_start(blin_sb[:], d_blin[:])
            ph = psum.tile([B, 512], F32, name="ph", tag="gps")
            for k in range(8):
                lhsT = hT[1][k % 2][:, 32 * (k // 2):32 * (k // 2) + 32]
                nc.tensor.matmul(ph[:], lhsT, wlin_sb[:, 512 * k:512 * (k + 1)],
                                 start=(k == 0), stop=(k == 7))
            out_sb = sb.tile([B, 512], F32, name="out_sb", tag="s_if")
            nc.vector.tensor_add(out_sb[:], ph[:], blin_sb[:])
            nc.sync.dma_start(d_y[:], out_sb[:])

    nc.compile()
    return nc, dict(res_list=res_list)


def _prep_common(inputs, meta):
    W_ih, W_hh = np.asarray(inputs["W_ih"]), np.asarray(inputs["W_hh"])
    b_sum = np.asarray(inputs["b_ih"]) + np.asarray(inputs["b_hh"])
    Wcat = np.concatenate([W_ih, W_hh], axis=2)               # [L, 4096, 2048]
    A = Wcat.reshape(L, 4, 4, 256, KT, 128)                   # [l, G, j, h2, k, p]
    Wt = np.ascontiguousarray(A.transpose(0, 4, 5, 2, 1, 3)   # [l, k, p, j, G, h2]
                              ).reshape(L, KT, 128, GCOLS)
    Wt = (Wt * SCALE).astype(ml_dtypes.float8_e3m4)
    Bf = b_sum.reshape(L, 4, 4, 256).transpose(0, 2, 1, 3)    # [l, j, G, h2]
    bias_fold = np.broadcast_to(Bf.reshape(L, 4, 1, 1024), (L, 4, 32, 1024))
    bias_fold = np.ascontiguousarray(bias_fold).reshape(L, 128, 1024)
    bias_fold = bias_fold.astype(ml_dtypes.bfloat16)
    wres = np.stack([Wt[l, k] for (l, k) in meta["res_list"]])
    W_lin = np.asarray(inputs["W_lin"]) * SCALE
    wlint = np.ascontiguousarray(
        W_lin.T.reshape(8, 128, 512).transpose(1, 0, 2).reshape(128, 8 * 512)
    ).astype(ml_dtypes.bfloat16)
    blin = np.ascontiguousarray(np.broadcast_to(
        np.asarray(inputs["b_lin"]).astype(np.float32)[None, :], (B, 512)))
    return dict(wres=wres, biasf=bias_fold, wlint=wlint, blinr=blin)


def _prep_core(inputs, ci):
    s = slice(ci * B, (ci + 1) * B)
    x = np.asarray(inputs["input_seq"])[s] * (1.0 / SCALE)
    h = np.asarray(inputs["h"])[:, s] * (1.0 / SCALE)
    c = np.asarray(inputs["c"])[:, s]
    x0t = np.ascontiguousarray(
        x.reshape(B, 8, 128).transpose(2, 1, 0).reshape(128, 8 * B)
    ).astype(ml_dtypes.bfloat16)
    hr = h.reshape(L, B, 4, 2, 128)
    h0t = np.ascontiguousarray(hr.transpose(0, 3, 4, 2, 1)
                               ).reshape(L, 2, 128, 128).astype(ml_dtypes.bfloat16)
    cr = c.reshape(L, B, 4, 256).transpose(0, 2, 1, 3)
    c0f = np.ascontiguousarray(cr).reshape(L, 128, 256).astype(np.float32)
    return dict(x0t=x0t, h0t=h0t, c0f=c0f)


_CACHE = {}


def _get_built():
    if "nc" not in _CACHE:
        _CACHE["nc"], _CACHE["meta"] = _build()
    return _CACHE["nc"], _CACHE["meta"]


def kernel(**inputs) -> np.ndarray:
    nc, meta = _get_built()
    common = _prep_common(inputs, meta)
    in_maps = [dict(common, **_prep_core(inputs, ci)) for ci in range(N_CORES)]
    r = bass_utils.run_bass_kernel_spmd(nc, in_maps, core_ids=list(range(N_CORES)))
    y = np.concatenate([r.results[ci]["y"] for ci in range(N_CORES)], axis=0)
    return y.astype(np.float32)


# revision 6
# speedup vs baseline: 1.7566x; 1.7566x over previous
"""TRN2 Bass kernel for nn_Decoder_25615184954195 (2-layer LSTM decoder, 32 steps).

Strategy: data-parallel over batch (8 cores x 32 rows), full sequential loop
local per core.  All 32 weight k-tiles are SBUF-resident in fp8 e3m4 (16 MB
per core), eliminating the per-step HBM weight streaming that bound the bf16
version.  Weights are pre-scaled by 128 into e3m4's normal range; the
stationary activations carry the compensating 1/128, folded for free into the
PSUM->SBUF copy after each h transpose (tensor_scalar_mul) and into the host
prep of x0/h0/W_lin.  PSUM accumulates in fp32 so gate values are exact up to
the e3m4 weight quantization (~2.4e-3 end-to-end rel l2).

Per-core layouts (unchanged from the bf16 version):
  - Contraction K = 2048 = [x(1024) | h(1024)], 16 k-tiles of 128.
  - Stationary operand: xhT k-tile [128, 32] bf16 (batch in the free dim).
  - Moving operand: column-permuted weight k-tile [128, 4096] fp8e3; the
    4-way column-tiled matmul's col-group j computes H-quarter j for all 4
    gates: psum[32*j + b, 256*G + h2] = gates[b, 1024*G + 256*j + h2].
  - Folded elementwise layout: partitions = (H-quarter, batch) so gate math
    runs on all 128 lanes.  c stays fp32.  h is rebuilt transposed via two
    PE transposes, deferred into the next layer-step between its h-rounds and
    x-rounds so the PE fills the gate-chain latency with matmuls.
"""
import sys

for _p in ("/opt/trn_rl_repo", "/root/.axon_site/_ro/trn_rl_repo"):
    if _p not in sys.path:
        sys.path.insert(0, _p)

import numpy as np
import ml_dtypes

import concourse.bacc as bacc
import concourse.mybir as mybir
import concourse.tile as tile
from concourse import masks
from concourse import bass_utils

BF16 = mybir.dt.bfloat16
FP8 = mybir.dt.float8e3
F32 = mybir.dt.float32
AF = mybir.ActivationFunctionType

N_CORES = 8
B = 32            # batch rows per core (256 total)
KT = 16           # k-tiles per layer
L = 2
GCOLS = 4096
NUM_STEPS = 32
SCALE = 128.0     # weights *SCALE in fp8; stationary activations /SCALE

ROUNDS = list(range(8, 16)) + list(range(0, 8))


def _build(num_steps=NUM_STEPS, reps=1, loop_reps=0):
    # weight tiles ordered by first use so step 0 starts ASAP
    res_list = [(l, k) for l in range(L) for k in ROUNDS]

    nc = bacc.Bacc("TRN2", target_bir_lowering=False, debug=False,
                   num_devices=N_CORES)

    d_w = nc.dram_tensor("wres", [len(res_list), 128, GCOLS], FP8,
                         kind="ExternalInput")
    d_bias = nc.dram_tensor("biasf", [L, 128, 1024], BF16, kind="ExternalInput")
    d_x0t = nc.dram_tensor("x0t", [128, 8 * B], BF16, kind="ExternalInput")
    d_h0t = nc.dram_tensor("h0t", [L, 2, 128, 128], BF16, kind="ExternalInput")
    d_c0 = nc.dram_tensor("c0f", [L, 128, 256], F32, kind="ExternalInput")
    d_wlin = nc.dram_tensor("wlint", [128, 8 * 512], BF16, kind="ExternalInput")
    d_blin = nc.dram_tensor("blinr", [B, 512], F32, kind="ExternalInput")
    d_y = nc.dram_tensor("y", [B, 512], F32, kind="ExternalOutput")

    with tile.TileContext(nc) as tc:
        import contextlib
        with contextlib.ExitStack() as ctx:
            sb = ctx.enter_context(tc.tile_pool(name="sb", bufs=1))
            sb2 = ctx.enter_context(tc.tile_pool(name="sb2", bufs=2))
            psum = ctx.enter_context(tc.tile_pool(name="psum", bufs=2, space="PSUM"))
            psum1 = ctx.enter_context(tc.tile_pool(name="psum1", bufs=2, space="PSUM"))
            psumgb = ctx.enter_context(tc.tile_pool(name="psumgb", bufs=1, space="PSUM"))

            w_sb = {}
            for i, (l, k) in enumerate(res_list):
                t = sb.tile([128, GCOLS], FP8, name=f"w_{l}_{k}")
                nc.sync.dma_start(t[:], d_w[i])
                w_sb[(l, k)] = t
            bias_sb = sb.tile([128, L * 1024], BF16, name="bias_sb")
            for l in range(L):
                nc.sync.dma_start(bias_sb[:, 1024 * l:1024 * (l + 1)], d_bias[l])
            x0t_sb = sb.tile([128, 8 * B], BF16, name="x0t_sb")
            nc.sync.dma_start(x0t_sb[:], d_x0t[:])
            hT = [[sb.tile([128, 128], BF16, name=f"hT_{l}_{hf}") for hf in range(2)]
                  for l in range(L)]
            for l in range(L):
                for hf in range(2):
                    nc.sync.dma_start(hT[l][hf][:], d_h0t[l, hf])
            c_sb = [sb.tile([128, 256], F32, name=f"c_{l}") for l in range(L)]
            for l in range(L):
                nc.sync.dma_start(c_sb[l][:], d_c0[l])
            wlin_sb = sb.tile([128, 8 * 512], BF16, name="wlin_sb")
            nc.sync.dma_start(wlin_sb[:], d_wlin[:])
            ident = sb.tile([128, 128], BF16, name="ident")
            masks.make_identity(nc, ident[:])

            def stat_ap(l, k, t):
                if k >= 8:
                    kh = k - 8
                    return hT[l][kh % 2][:, 32 * (kh // 2):32 * (kh // 2) + 32]
                if l == 0:
                    if t == 0:
                        return x0t_sb[:, 32 * k:32 * (k + 1)]
                    src = hT[1]
                else:
                    src = hT[0]
                return src[k % 2][:, 32 * (k // 2):32 * (k // 2) + 32]

            pending_tr = []   # deferred transposes: (hbf tile, target layer)

            def flush_transposes():
                while pending_tr:
                    hbf_p, l_p = pending_tr.pop(0)
                    for hf in range(2):
                        pt = psum1.tile([128, 128], BF16, name="pt", tag="pt")
                        nc.tensor.transpose(pt[:], hbf_p[:, 128 * hf:128 * (hf + 1)],
                                            ident[:])
                        nc.vector.tensor_scalar_mul(hT[l_p][hf][:], pt[:], 1.0 / SCALE)

            def emit_step(tt):
                    for l in range(L):
                        ps = psum.tile([128, 1024], F32, name="gps", tag="gps")
                        for ri, k in enumerate(ROUNDS):
                            if ri == 8:
                                flush_transposes()   # before x-rounds need hT
                            w = w_sb[(l, k)]
                            lhsT = stat_ap(l, k, tt)
                            for hv in range(2):      # PSUM bank limit: N<=512 fp32
                                for j in range(4):   # col-groups run concurrently
                                    nc.tensor.matmul(
                                        ps[32 * j:32 * (j + 1), 512 * hv:512 * (hv + 1)],
                                        lhsT,
                                        w[:, 1024 * j + 512 * hv:1024 * j + 512 * (hv + 1)],
                                        start=(ri == 0), stop=(ri == len(ROUNDS) - 1),
                                        tile_position=(0, 32 * j),
                                        skip_group_check=True,
                                    )

                        gb = psumgb.tile([128, 1024], F32, name="gb", tag="gb")
                        nc.vector.tensor_add(gb[:], ps[:],
                                             bias_sb[:, 1024 * l:1024 * (l + 1)])
                        s_if = sb.tile([128, 512], F32, name="s_if", tag="s_if")
                        t_g = sb.tile([128, 256], F32, name="t_g", tag="t_g")
                        s_o = sb.tile([128, 256], F32, name="s_o", tag="s_o")
                        nc.scalar.activation(s_if[:], gb[:, 0:512], AF.Sigmoid)
                        nc.scalar.activation(t_g[:], gb[:, 512:768], AF.Tanh)
                        nc.scalar.activation(s_o[:], gb[:, 768:1024], AF.Sigmoid)
                        t1 = sb.tile([128, 256], F32, name="t1", tag="t1")
                        t2 = sb.tile([128, 256], F32, name="t2", tag="t2")
                        nc.vector.tensor_mul(t1[:], s_if[:, 256:512], c_sb[l][:])
                        nc.vector.tensor_mul(t2[:], s_if[:, 0:256], t_g[:])
                        nc.vector.tensor_add(c_sb[l][:], t1[:], t2[:])
                        th = sb.tile([128, 256], F32, name="th", tag="t1")
                        nc.scalar.activation(th[:], c_sb[l][:], AF.Tanh)
                        hbf = sb2.tile([128, 256], BF16, name="hbf", tag="hbf")
                        nc.vector.tensor_mul(hbf[:], s_o[:], th[:])
                        pending_tr.append((hbf, l))

            if loop_reps > 0:
                # device-side repeat loop for timing: steady-state workload
                # (every step reads hT, transposes flushed at body end so the
                # body is iteration-self-contained; all-engine barrier on the
                # back edge handles loop-carried state)
                with tc.For_i(0, loop_reps):
                    for t in range(num_steps):
                        emit_step(1)
                    flush_transposes()
            else:
                for rep in range(reps):
                    for t in range(num_steps):
                        # steps past the first read hT, not x0
                        emit_step(t if rep == 0 else 1)
                flush_transposes()

            # ---- linear head: pred = h1 @ W_lin.T + b_lin ----
            blin_sb = sb.tile([B, 512], F32, name="blin_sb", tag="t1")
            nc.sync.dma_start(blin_sb[:], d_blin[:])
            ph = psum.tile([B, 512], F32, name="ph", tag="gps")
            for k in range(8):
                lhsT = hT[1][k % 2][:, 32 * (k // 2):32 * (k // 2) + 32]
                nc.tensor.matmul(ph[:], lhsT, wlin_sb[:, 512 * k:512 * (k + 1)],
                                 start=(k == 0), stop=(k == 7))
            out_sb = sb.tile([B, 512], F32, name="out_sb", tag="s_if")
            nc.vector.tensor_add(out_sb[:], ph[:], blin_sb[:])
            nc.sync.dma_start(d_y[:], out_sb[:])

    nc.compile()
    return nc, dict(res_list=res_list)


def _prep_common(inputs, meta):
    W_ih, W_hh = np.asarray(inputs["W_ih"]), np.asarray(inputs["W_hh"])
    b_sum = np.asarray(inputs["b_ih"]) + np.asarray(inputs["b_hh"])
    Wcat = np.concatenate([W_ih, W_hh], axis=2)               # [L, 4096, 2048]
    A = Wcat.reshape(L, 4, 4, 256, KT, 128)                   # [l, G, j, h2, k, p]
    Wt = np.ascontiguousarray(A.transpose(0, 4, 5, 2, 1, 3)   # [l, k, p, j, G, h2]
                              ).reshape(L, KT, 128, GCOLS)
    Wt = (Wt * SCALE).astype(ml_dtypes.float8_e3m4)
    Bf = b_sum.reshape(L, 4, 4, 256).transpose(0, 2, 1, 3)    # [l, j, G, h2]
    bias_fold = np.broadcast_to(Bf.reshape(L, 4, 1, 1024), (L, 4, 32, 1024))
    bias_fold = np.ascontiguousarray(bias_fold).reshape(L, 128, 1024)
    bias_fold = bias_fold.astype(ml_dtypes.bfloat16)
    wres = np.stack([Wt[l, k] for (l, k) in meta["res_list"]])
    W_lin = np.asarray(inputs["W_lin"]) * SCALE
    wlint = np.ascontiguousarray(
        W_lin.T.reshape(8, 128, 512).transpose(1, 0, 2).reshape(128, 8 * 512)
    ).astype(ml_dtypes.bfloat16)
    blin = np.ascontiguousarray(np.broadcast_to(
        np.asarray(inputs["b_lin"]).astype(np.float32)[None, :], (B, 512)))
    return dict(wres=wres, biasf=bias_fold, wlint=wlint, blinr=blin)


def _prep_core(inputs, ci):
    s = slice(ci * B, (ci + 1) * B)
    x = np.asarray(inputs["input_seq"])[s] * (1.0 / SCALE)
    h = np.asarray(inputs["h"])[:, s] * (1.0 / SCALE)
    c = np.asarray(inputs["c"])[:, s]
    x0t = np.ascontiguousarray(
        x.reshape(B, 8, 128).transpose(2, 1, 0).reshape(128, 8 * B)
    ).astype(ml_dtypes.bfloat16)
    hr = h.reshape(L, B, 4, 2, 128)
    h0t = np.ascontiguousarray(hr.transpose(0, 3, 4, 2, 1)
                               ).reshape(L, 2, 128, 128).astype(ml_dtypes.bfloat16)
    cr = c.reshape(L, B, 4, 256).transpose(0, 2, 1, 3)
    c0f = np.ascontiguousarray(cr).reshape(L, 128, 256).astype(np.float32)
    return dict(x0t=x0t, h0t=h0t, c0f=c0f)


_CACHE = {}


def _get_built():
    if "nc" not in _CACHE:
        _CACHE["nc"], _CACHE["meta"] = _build()
    return _CACHE["nc"], _CACHE["meta"]


def kernel(**inputs) -> np.ndarray:
    nc, meta = _get_built()
    common = _prep_common(inputs, meta)
    in_maps = [dict(common, **_prep_core(inputs, ci)) for ci in range(N_CORES)]
    r = bass_utils.run_bass_kernel_spmd(nc, in_maps, core_ids=list(range(N_CORES)))
    y = np.concatenate([r.results[ci]["y"] for ci in range(N_CORES)], axis=0)
    return y.astype(np.float32)
